# revision 39
# baseline (speedup 1.0000x reference)
"""Trainium2 Bass kernel for nn_Blur: 4x4 FIR depthwise blur with pad (2,1).

out[n,c,i,j] = sum_{a,b} K[a,b] * x[n,c, i+1-a, j+1-b]   (zero-padded)

Strategy (8 NeuronCores, pure data parallelism over the 8192 (n,c) slices,
bf16 I/O to halve HBM traffic — the 2e-2 gate leaves ~5x margin):
  - Each core processes 1024 slices of 64x64, 8 SBUF tiles of 128 slices.
  - W-parity packing: partition p = wp*64 + u (wp = w%2, u = h), free =
    (gc, slice) with gc = 1 + w//2; the two zero pad columns (gc=0, 33)
    are memset once per pool buffer and never shipped over HBM.
    The 4 W-taps of an output column then span only THREE gc columns, so
    the whole 4x4 blur is 3 PSUM-accumulated matmuls (vs 4 for the
    member-packed layout): lhsT_sh[(wp,u),(jp,i)] = K[i+1-u, jp-wp+1-2sh].
  - bf16 matmuls run at 1 col/cycle; weights (1/16, 3/16, 9/16 scale) are
    exact in bf16, accumulation is fp32 in PSUM.
  - PSUM->SBUF copies cast fp32->bf16 and alternate DVE/ACT engines.
  - Loads ride the SP HWDGE ring, stores the ACT ring: neither in-order
    sequencer ever head-of-line blocks the other's semaphore waits.
  - PE warm-up matmuls on an on-chip memset tile open the HAM clock gate
    (0.65/1.2 -> 2.4 GHz) before the first real matmul.
  - The host pre-permutes each core's shard into the exact SBUF tile
    layout, so every DMA descriptor is one contiguous 8KB run/partition.
"""

import sys
import types

import numpy as np
import ml_dtypes

import concourse.bacc as bacc
import concourse.mybir as mybir
from concourse.tile import TileContext
from concourse.bass_utils import run_bass_kernel_spmd

BF16_NP = ml_dtypes.bfloat16


def _install_ntff_hook():
    """Best-effort shim: this image's antenv lacks axon_hooks, which the
    trace=True path of run_bass_kernel_spmd imports. Harmless if unused."""
    if "antenv.axon_hooks" in sys.modules:
        return
    try:
        sys.path.insert(0, "/root/.axon_site")
        from trn_agent_boot.trn_boot import _ntff_profile_via_ctypes

        hook = _ntff_profile_via_ctypes("/opt/axon/libaxon_pjrt.so")
        mod = types.ModuleType("antenv.axon_hooks")
        mod.get_axon_ntff_profile_hook = lambda: hook
        mod.set_axon_ntff_profile_hook = lambda h: None
        sys.modules["antenv.axon_hooks"] = mod
    except Exception:
        pass


_install_ntff_hook()

N_CORES = 8
B, C, H, W = 32, 256, 64, 64
NSLICES = B * C                       # 8192
SLICES_PER_CORE = NSLICES // N_CORES  # 1024
TILE_SLICES = 64                      # slices per SBUF tile
G = W // 2                            # 32 w-parity column groups
GC = G + 2                            # + zero pad col on each side
QS = 16                               # slices per PSUM group (16*32 = 512;
                                      # walrus ISA caps a matmul dst at one
                                      # 2KB PSUM bank)
XBUFS = 8                             # input-tile ring depth
F32 = mybir.dt.float32
BF16 = mybir.dt.bfloat16

_NC_CACHE = {}


def _build_wmat(K: np.ndarray) -> np.ndarray:
    """(128, 512) bf16: per-shift stationary matrices in SBUF layout.

    lhsT_sh[(wp,u), (jp,i)] = K[a, b] with a = i+1-u, b = jp-wp+1-2*sh,
    for shifts sh in (-1, 0, +1); entries with a or b outside 0..3 are 0.
    Pre-transposed to [k, (sh, m)] so the weight DMA is one contiguous
    1KB run per partition. 4th slot is zero padding.
    """
    K = np.asarray(K, np.float32)
    wmat = np.zeros((4, 128, 128), np.float32)
    for si, sh in enumerate((-1, 0, 1)):
        for wp in range(2):
            for jp in range(2):
                b = jp - wp + 1 - 2 * sh
                if not 0 <= b <= 3:
                    continue
                T = np.zeros((H, H), np.float32)
                for i in range(H):
                    for u in range(max(0, i - 2), min(H, i + 2)):
                        T[u, i] = K[i + 1 - u, b]
                wmat[si, wp * 64 : wp * 64 + 64, jp * 64 : jp * 64 + 64] = T
    return np.ascontiguousarray(
        wmat.transpose(1, 0, 2).reshape(128, 4 * 128)
    ).astype(BF16_NP)


WARMUP_MMS = 4


def _build_nc(slices_per_core: int = SLICES_PER_CORE):
    ntiles = slices_per_core // TILE_SLICES
    nc = bacc.Bacc("TRN2", target_bir_lowering=False, debug=False)
    # DRAM layouts are the SBUF tile layouts (host pre-/post-permutes):
    #   x: [tile, p=(wp u), (g s)]  (no pad columns — memset on chip)
    #   y: [tile, p=(jp i), (g s)]
    nfull = ntiles - 2
    HTS = TILE_SLICES // 2
    x = nc.dram_tensor(
        "x", [nfull, 128, G * TILE_SLICES], BF16, kind="ExternalInput"
    ).ap()
    # first + last tiles each ship as two half tiles: the first 0.25MB load
    # lands ~1us earlier (real matmuls start sooner), and the final
    # (compute-gated) store is 0.25MB instead of 0.5MB (~1us faster drain)
    x2 = nc.dram_tensor(
        "x2", [4, 128, G * HTS], BF16, kind="ExternalInput"
    ).ap()
    wm = nc.dram_tensor("w", [128, 4 * 128], BF16, kind="ExternalInput").ap()
    y = nc.dram_tensor(
        "y", [nfull, 128, G * TILE_SLICES], BF16, kind="ExternalOutput"
    ).ap()
    y2 = nc.dram_tensor(
        "y2", [4, 128, G * HTS], BF16, kind="ExternalOutput"
    ).ap()
    # sink for the PE warm-up matmuls (kept alive so DCE can't drop them)
    warm_out = nc.dram_tensor("warm", [128, 4], F32, kind="ExternalOutput").ap()

    with TileContext(nc) as tc:
        with (
            tc.tile_pool(name="wpool", bufs=1) as wpool,
            tc.tile_pool(name="xpool", bufs=1) as xpool,
            tc.tile_pool(name="opool", bufs=4) as opool,
            tc.tile_pool(name="pspool", bufs=8, space="PSUM") as pspool,
        ):
            # weights ride the ACT ring; the SP ring issues ONLY the
            # input-tile loads so prefetch is never head-of-line blocked
            # behind a store's semaphore wait (in-order sequencer)
            wsb = wpool.tile([128, 4, 128], BF16, name="wsb")
            nc.scalar.dma_start(wsb.rearrange("k b m -> k (b m)"), wm)

            # PE warm-up source tile memset on the DVE queue (starts
            # earliest after the preamble; gpsimd keeps the pad memsets)
            wz = wpool.tile([128, 512], BF16, name="wz")
            nc.vector.memset(wz[:], 0)

            # input ring: manual buffer list so the two pad columns are
            # memset exactly once per buffer (the per-tile DMA only ever
            # rewrites the middle 32 columns)
            xts = []
            for i in range(XBUFS):
                xt = xpool.tile([128, GC, TILE_SLICES], BF16, name=f"xt{i}")
                nc.gpsimd.memset(xt[:, 0, :], 0)
                nc.gpsimd.memset(xt[:, GC - 1, :], 0)
                xts.append(xt)
            xhs = []
            for i in range(4):
                xh = xpool.tile([128, GC, HTS], BF16, name=f"xh{i}")
                nc.gpsimd.memset(xh[:, 0, :], 0)
                nc.gpsimd.memset(xh[:, GC - 1, :], 0)
                xhs.append(xh)

            # PE warm-up (no DMA dependency): the HAM clock gate needs
            # ~3us of continuous PE activity to open (0.65/1.2 -> 2.4 GHz)
            # before the real matmuls start; sized to end right as tile0's
            # load lands so the PE never idles (an idle gap would drop the
            # clock back to the mid p-state).
            wscratch = wpool.tile([128, 4], F32, name="wscratch")
            wps = pspool.tile([128, G, QS], F32, name="wps", tag="ps")
            for r in range(WARMUP_MMS):
                nc.tensor.matmul(
                    wps[:],
                    wz[:, 0:128],
                    wz[:],
                    start=(r == 0),
                    stop=(r == WARMUP_MMS - 1),
                )

            nq = TILE_SLICES // QS

            def half_unit(hti, last):
                xh = xhs[hti]
                nc.sync.dma_start(xh[:, 1 : 1 + G, :], x2[hti])
                oh = opool.tile([128, G, HTS], BF16, name="oh")
                for q in range(HTS // QS):
                    ps = pspool.tile([128, G, QS], F32, name="ps")
                    for si in range(3):
                        nc.tensor.matmul(
                            ps[:],
                            wsb[:, si, :],
                            xh[:, si : si + G, QS * q : QS * q + QS],
                            start=(si == 0),
                            stop=(si == 2),
                        )
                    if last and q == HTS // QS - 1:
                        # final copy split across BOTH engines: halves the
                        # last serial PSUM->SBUF hop before the final store
                        h0 = QS * q
                        nc.vector.tensor_copy(
                            oh[:, :, h0 : h0 + QS // 2], ps[:, :, : QS // 2]
                        )
                        nc.scalar.copy(
                            oh[:, :, h0 + QS // 2 : h0 + QS],
                            ps[:, :, QS // 2 :],
                        )
                    elif q % 2 == 0:
                        nc.vector.tensor_copy(
                            oh[:, :, QS * q : QS * q + QS], ps[:]
                        )
                    else:
                        nc.scalar.copy(oh[:, :, QS * q : QS * q + QS], ps[:])
                # the very last store rides the (now idle) SP ring so its
                # descriptor write overlaps the ACT ring's prior store
                store_eng = nc.sync if last else nc.scalar
                store_eng.dma_start(y2[hti], oh[:])

            for hti in range(2):
                half_unit(hti, last=False)

            for t in range(nfull):
                xt = xts[t % XBUFS]
                nc.sync.dma_start(xt[:, 1 : 1 + G, :], x[t])

                # one output tile per input tile; psum-group copies fill it
                ot = opool.tile([128, G, TILE_SLICES], BF16, name="ot")
                for q in range(nq):
                    ps = pspool.tile([128, G, QS], F32, name="ps")
                    for si in range(3):
                        nc.tensor.matmul(
                            ps[:],
                            wsb[:, si, :],
                            xt[:, si : si + G, QS * q : QS * q + QS],
                            start=(si == 0),
                            stop=(si == 2),
                        )
                    # alternate copy engine: DVE and ACT share the load
                    if q % 2 == 0:
                        nc.vector.tensor_copy(
                            ot[:, :, QS * q : QS * q + QS], ps[:]
                        )
                    else:
                        nc.scalar.copy(ot[:, :, QS * q : QS * q + QS], ps[:])
                    if t == 0 and q == 0:
                        # emitted here so its sequencer slot never blocks
                        # tile copies; frees the warmup psum slot
                        nc.vector.tensor_copy(wscratch[:], wps[:, 0, 0:4])

                # single whole-tile store on the ACT ring: sequencer
                # descriptor-write cost stays off the per-psum-group
                # critical path and off the SP load-prefetch ring
                st_eng = nc.sync if t == nfull - 1 else nc.scalar
                st_eng.dma_start(y[t], ot[:])
                if t == 1:
                    # warm-up sink store hidden mid-stream (defeats DCE
                    # without extending the ACT ring past the last y store)
                    nc.scalar.dma_start(warm_out, wscratch[:])

            half_unit(2, last=False)
            half_unit(3, last=True)

    nc.compile()
    return nc


def get_nc(slices_per_core: int = SLICES_PER_CORE):
    if slices_per_core not in _NC_CACHE:
        _NC_CACHE[slices_per_core] = _build_nc(slices_per_core)
    return _NC_CACHE[slices_per_core]


def _pack_block(xs: np.ndarray, ts: int) -> np.ndarray:
    """[S, H, W] fp32 -> [S/ts, 128, 32*ts] bf16 in SBUF tile layout."""
    nt = xs.shape[0] // ts
    # (t, s, u, g, wp) -> (t, wp, u, g, s)
    v = xs.reshape(nt, ts, H, G, 2).transpose(0, 4, 2, 3, 1)
    return np.ascontiguousarray(v.reshape(nt, 128, G * ts)).astype(BF16_NP)


def _unpack_block(yp: np.ndarray, ts: int) -> np.ndarray:
    """[nt, 128, 32*ts] bf16 -> [nt*ts, H, W] fp32."""
    nt = yp.shape[0]
    # [(jp, i), (g, s)] -> [s, i, (g, jp)]
    v = yp.reshape(nt, 2, H, G, ts).transpose(0, 4, 2, 3, 1)
    return v.reshape(nt * ts, H, W).astype(np.float32)


def kernel(x: np.ndarray, kernel: np.ndarray, _trace: bool = False, **_tkw):
    x = np.asarray(x, np.float32)
    wmat = _build_wmat(kernel)
    b, c, h, w = x.shape
    xs = x.reshape(b * c, h, w)
    spc = (b * c) // N_CORES
    nc = get_nc(spc)
    hts = TILE_SLICES // 2
    in_maps = []
    for k in range(N_CORES):
        sl = xs[k * spc : (k + 1) * spc]
        in_maps.append(
            {
                "x": _pack_block(sl[TILE_SLICES : spc - TILE_SLICES], TILE_SLICES),
                "x2": np.concatenate(
                    [
                        _pack_block(sl[:TILE_SLICES], hts),
                        _pack_block(sl[spc - TILE_SLICES :], hts),
                    ],
                    axis=0,
                ),
                "w": wmat,
            }
        )
    res = run_bass_kernel_spmd(
        nc, in_maps, list(range(N_CORES)), trace=_trace, **_tkw
    )
    hts2 = TILE_SLICES // 2
    out = np.concatenate(
        [
            np.concatenate(
                [
                    _unpack_block(res.results[k]["y2"][:2], hts2),
                    _unpack_block(res.results[k]["y"], TILE_SLICES),
                    _unpack_block(res.results[k]["y2"][2:], hts2),
                ],
                axis=0,
            )
            for k in range(N_CORES)
        ],
        axis=0,
    )
    result = out.reshape(b, c, h, w)
    if _trace:
        return result, res
    return result


# revision 40
# speedup vs baseline: 1.1243x; 1.1243x over previous
"""Trainium2 Bass kernel for nn_Blur: 4x4 FIR depthwise blur with pad (2,1).

out[n,c,i,j] = sum_{a,b} K[a,b] * x[n,c, i+1-a, j+1-b]   (zero-padded)

Strategy (8 NeuronCores, pure data parallelism over the 8192 (n,c) slices,
bf16 I/O to halve HBM traffic — the 2e-2 gate leaves ~5x margin):
  - Each core processes 1024 slices of 64x64, 8 SBUF tiles of 128 slices.
  - W-parity packing: partition p = wp*64 + u (wp = w%2, u = h), free =
    (gc, slice) with gc = 1 + w//2; the two zero pad columns (gc=0, 33)
    are memset once per pool buffer and never shipped over HBM.
    The 4 W-taps of an output column then span only THREE gc columns, so
    the whole 4x4 blur is 3 PSUM-accumulated matmuls (vs 4 for the
    member-packed layout): lhsT_sh[(wp,u),(jp,i)] = K[i+1-u, jp-wp+1-2sh].
  - bf16 matmuls run at 1 col/cycle; weights (1/16, 3/16, 9/16 scale) are
    exact in bf16, accumulation is fp32 in PSUM.
  - PSUM->SBUF copies cast fp32->bf16 and alternate DVE/ACT engines.
  - Loads ride the SP HWDGE ring, stores the ACT ring: neither in-order
    sequencer ever head-of-line blocks the other's semaphore waits.
  - PE warm-up matmuls on an on-chip memset tile open the HAM clock gate
    (0.65/1.2 -> 2.4 GHz) before the first real matmul.
  - The host pre-permutes each core's shard into the exact SBUF tile
    layout, so every DMA descriptor is one contiguous 8KB run/partition.
"""

import sys
import types

import numpy as np
import ml_dtypes

import concourse.bacc as bacc
import concourse.mybir as mybir
from concourse.tile import TileContext
from concourse.bass_utils import run_bass_kernel_spmd

BF16_NP = ml_dtypes.bfloat16


def _install_ntff_hook():
    """Best-effort shim: this image's antenv lacks axon_hooks, which the
    trace=True path of run_bass_kernel_spmd imports. Harmless if unused."""
    if "antenv.axon_hooks" in sys.modules:
        return
    try:
        sys.path.insert(0, "/root/.axon_site")
        from trn_agent_boot.trn_boot import _ntff_profile_via_ctypes

        hook = _ntff_profile_via_ctypes("/opt/axon/libaxon_pjrt.so")
        mod = types.ModuleType("antenv.axon_hooks")
        mod.get_axon_ntff_profile_hook = lambda: hook
        mod.set_axon_ntff_profile_hook = lambda h: None
        sys.modules["antenv.axon_hooks"] = mod
    except Exception:
        pass


_install_ntff_hook()

N_CORES = 8
B, C, H, W = 32, 256, 64, 64
NSLICES = B * C                       # 8192
SLICES_PER_CORE = NSLICES // N_CORES  # 1024
TILE_SLICES = 64                      # slices per SBUF tile
G = W // 2                            # 32 w-parity column groups
GC = G + 2                            # + zero pad col on each side
QS = 16                               # slices per PSUM group (16*32 = 512;
                                      # walrus ISA caps a matmul dst at one
                                      # 2KB PSUM bank)
XBUFS = 8                             # input-tile ring depth
F32 = mybir.dt.float32
BF16 = mybir.dt.bfloat16

_NC_CACHE = {}


def _build_wmat(K: np.ndarray) -> np.ndarray:
    """(128, 512) bf16: per-shift stationary matrices in SBUF layout.

    lhsT_sh[(wp,u), (jp,i)] = K[a, b] with a = i+1-u, b = jp-wp+1-2*sh,
    for shifts sh in (-1, 0, +1); entries with a or b outside 0..3 are 0.
    Pre-transposed to [k, (sh, m)] so the weight DMA is one contiguous
    1KB run per partition. 4th slot is zero padding.
    """
    K = np.asarray(K, np.float32)
    wmat = np.zeros((4, 128, 128), np.float32)
    for si, sh in enumerate((-1, 0, 1)):
        for wp in range(2):
            for jp in range(2):
                b = jp - wp + 1 - 2 * sh
                if not 0 <= b <= 3:
                    continue
                T = np.zeros((H, H), np.float32)
                for i in range(H):
                    for u in range(max(0, i - 2), min(H, i + 2)):
                        T[u, i] = K[i + 1 - u, b]
                wmat[si, wp * 64 : wp * 64 + 64, jp * 64 : jp * 64 + 64] = T
    return np.ascontiguousarray(
        wmat.transpose(1, 0, 2).reshape(128, 4 * 128)
    ).astype(BF16_NP)


WARMUP_MMS = 5


def _build_nc(slices_per_core: int = SLICES_PER_CORE):
    ntiles = slices_per_core // TILE_SLICES
    nc = bacc.Bacc("TRN2", target_bir_lowering=False, debug=False)
    # DRAM layouts are the SBUF tile layouts (host pre-/post-permutes):
    #   x: [tile, p=(wp u), (g s)]  (no pad columns — memset on chip)
    #   y: [tile, p=(jp i), (g s)]
    nfull = ntiles - 2
    HTS = TILE_SLICES // 2
    x = nc.dram_tensor(
        "x", [nfull, 128, G * TILE_SLICES], BF16, kind="ExternalInput"
    ).ap()
    # first + last tiles each ship as two half tiles: the first 0.25MB load
    # lands ~1us earlier (real matmuls start sooner), and the final
    # (compute-gated) store is 0.25MB instead of 0.5MB (~1us faster drain)
    x2 = nc.dram_tensor(
        "x2", [4, 128, G * HTS], BF16, kind="ExternalInput"
    ).ap()
    wm = nc.dram_tensor("w", [128, 4 * 128], BF16, kind="ExternalInput").ap()
    y = nc.dram_tensor(
        "y", [nfull, 128, G * TILE_SLICES], BF16, kind="ExternalOutput"
    ).ap()
    y2 = nc.dram_tensor(
        "y2", [4, 128, G * HTS], BF16, kind="ExternalOutput"
    ).ap()
    # sink for the PE warm-up matmuls (kept alive so DCE can't drop them)
    warm_out = nc.dram_tensor("warm", [128, 4], F32, kind="ExternalOutput").ap()

    with TileContext(nc) as tc:
        with (
            tc.tile_pool(name="wpool", bufs=1) as wpool,
            tc.tile_pool(name="xpool", bufs=1) as xpool,
            tc.tile_pool(name="opool", bufs=4) as opool,
            tc.tile_pool(name="pspool", bufs=8, space="PSUM") as pspool,
        ):
            # weights ride the ACT ring; the SP ring issues ONLY the
            # input-tile loads so prefetch is never head-of-line blocked
            # behind a store's semaphore wait (in-order sequencer)
            wsb = wpool.tile([128, 4, 128], BF16, name="wsb")
            nc.scalar.dma_start(wsb.rearrange("k b m -> k (b m)"), wm)

            # PE warm-up source tile memset on the DVE queue (starts
            # earliest after the preamble; gpsimd keeps the pad memsets)
            wz = wpool.tile([128, 512], BF16, name="wz")
            nc.vector.memset(wz[:], 0)

            # input ring: manual buffer list so the two pad columns are
            # memset exactly once per buffer (the per-tile DMA only ever
            # rewrites the middle 32 columns)
            xts = []
            for i in range(XBUFS):
                xt = xpool.tile([128, GC, TILE_SLICES], BF16, name=f"xt{i}")
                nc.gpsimd.memset(xt[:, 0, :], 0)
                nc.gpsimd.memset(xt[:, GC - 1, :], 0)
                xts.append(xt)
            xhs = []
            for i in range(4):
                xh = xpool.tile([128, GC, HTS], BF16, name=f"xh{i}")
                nc.gpsimd.memset(xh[:, 0, :], 0)
                nc.gpsimd.memset(xh[:, GC - 1, :], 0)
                xhs.append(xh)

            # PE warm-up (no DMA dependency): the HAM clock gate needs
            # ~3us of continuous PE activity to open (0.65/1.2 -> 2.4 GHz)
            # before the real matmuls start; sized to end right as tile0's
            # load lands so the PE never idles (an idle gap would drop the
            # clock back to the mid p-state).
            wscratch = wpool.tile([128, 4], F32, name="wscratch")
            wps = pspool.tile([128, G, QS], F32, name="wps", tag="ps")
            for r in range(WARMUP_MMS):
                nc.tensor.matmul(
                    wps[:],
                    wz[:, 0:128],
                    wz[:],
                    start=(r == 0),
                    stop=(r == WARMUP_MMS - 1),
                )

            nq = TILE_SLICES // QS

            def half_unit(hti, last):
                xh = xhs[hti]
                nc.sync.dma_start(xh[:, 1 : 1 + G, :], x2[hti])
                oh = opool.tile([128, G, HTS], BF16, name="oh")
                for q in range(HTS // QS):
                    ps = pspool.tile([128, G, QS], F32, name="ps")
                    for si in range(3):
                        nc.tensor.matmul(
                            ps[:],
                            wsb[:, si, :],
                            xh[:, si : si + G, QS * q : QS * q + QS],
                            start=(si == 0),
                            stop=(si == 2),
                        )
                    if last and q == HTS // QS - 1:
                        # final copy split across BOTH engines: halves the
                        # last serial PSUM->SBUF hop before the final store
                        h0 = QS * q
                        nc.vector.tensor_copy(
                            oh[:, :, h0 : h0 + QS // 2], ps[:, :, : QS // 2]
                        )
                        nc.scalar.copy(
                            oh[:, :, h0 + QS // 2 : h0 + QS],
                            ps[:, :, QS // 2 :],
                        )
                    elif q % 2 == 0:
                        nc.vector.tensor_copy(
                            oh[:, :, QS * q : QS * q + QS], ps[:]
                        )
                    else:
                        nc.scalar.copy(oh[:, :, QS * q : QS * q + QS], ps[:])
                # the very last store rides the (now idle) SP ring so its
                # descriptor write overlaps the ACT ring's prior store
                store_eng = nc.sync if last else nc.scalar
                store_eng.dma_start(y2[hti], oh[:])

            for hti in range(2):
                half_unit(hti, last=False)

            for t in range(nfull):
                xt = xts[t % XBUFS]
                nc.sync.dma_start(xt[:, 1 : 1 + G, :], x[t])

                # one output tile per input tile; psum-group copies fill it
                ot = opool.tile([128, G, TILE_SLICES], BF16, name="ot")
                for q in range(nq):
                    ps = pspool.tile([128, G, QS], F32, name="ps")
                    for si in range(3):
                        nc.tensor.matmul(
                            ps[:],
                            wsb[:, si, :],
                            xt[:, si : si + G, QS * q : QS * q + QS],
                            start=(si == 0),
                            stop=(si == 2),
                        )
                    # alternate copy engine: DVE and ACT share the load
                    if q % 2 == 0:
                        nc.vector.tensor_copy(
                            ot[:, :, QS * q : QS * q + QS], ps[:]
                        )
                    else:
                        nc.scalar.copy(ot[:, :, QS * q : QS * q + QS], ps[:])
                    if t == 0 and q == 0:
                        # emitted here so its sequencer slot never blocks
                        # tile copies; frees the warmup psum slot
                        nc.vector.tensor_copy(wscratch[:], wps[:, 0, 0:4])

                # single whole-tile store on the ACT ring: sequencer
                # descriptor-write cost stays off the per-psum-group
                # critical path and off the SP load-prefetch ring
                nc.scalar.dma_start(y[t], ot[:])
                if t == 1:
                    # warm-up sink store hidden mid-stream (defeats DCE
                    # without extending the ACT ring past the last y store)
                    nc.scalar.dma_start(warm_out, wscratch[:])

            half_unit(2, last=False)
            half_unit(3, last=True)

    nc.compile()
    return nc


def get_nc(slices_per_core: int = SLICES_PER_CORE):
    if slices_per_core not in _NC_CACHE:
        _NC_CACHE[slices_per_core] = _build_nc(slices_per_core)
    return _NC_CACHE[slices_per_core]


def _pack_block(xs: np.ndarray, ts: int) -> np.ndarray:
    """[S, H, W] fp32 -> [S/ts, 128, 32*ts] bf16 in SBUF tile layout."""
    nt = xs.shape[0] // ts
    # (t, s, u, g, wp) -> (t, wp, u, g, s)
    v = xs.reshape(nt, ts, H, G, 2).transpose(0, 4, 2, 3, 1)
    return np.ascontiguousarray(v.reshape(nt, 128, G * ts)).astype(BF16_NP)


def _unpack_block(yp: np.ndarray, ts: int) -> np.ndarray:
    """[nt, 128, 32*ts] bf16 -> [nt*ts, H, W] fp32."""
    nt = yp.shape[0]
    # [(jp, i), (g, s)] -> [s, i, (g, jp)]
    v = yp.reshape(nt, 2, H, G, ts).transpose(0, 4, 2, 3, 1)
    return v.reshape(nt * ts, H, W).astype(np.float32)


def kernel(x: np.ndarray, kernel: np.ndarray, _trace: bool = False, **_tkw):
    x = np.asarray(x, np.float32)
    wmat = _build_wmat(kernel)
    b, c, h, w = x.shape
    xs = x.reshape(b * c, h, w)
    spc = (b * c) // N_CORES
    nc = get_nc(spc)
    hts = TILE_SLICES // 2
    in_maps = []
    for k in range(N_CORES):
        sl = xs[k * spc : (k + 1) * spc]
        in_maps.append(
            {
                "x": _pack_block(sl[TILE_SLICES : spc - TILE_SLICES], TILE_SLICES),
                "x2": np.concatenate(
                    [
                        _pack_block(sl[:TILE_SLICES], hts),
                        _pack_block(sl[spc - TILE_SLICES :], hts),
                    ],
                    axis=0,
                ),
                "w": wmat,
            }
        )
    res = run_bass_kernel_spmd(
        nc, in_maps, list(range(N_CORES)), trace=_trace, **_tkw
    )
    hts2 = TILE_SLICES // 2
    out = np.concatenate(
        [
            np.concatenate(
                [
                    _unpack_block(res.results[k]["y2"][:2], hts2),
                    _unpack_block(res.results[k]["y"], TILE_SLICES),
                    _unpack_block(res.results[k]["y2"][2:], hts2),
                ],
                axis=0,
            )
            for k in range(N_CORES)
        ],
        axis=0,
    )
    result = out.reshape(b, c, h, w)
    if _trace:
        return result, res
    return result


# revision 42
# speedup vs baseline: 1.1718x; 1.0423x over previous
"""Trainium2 Bass kernel for nn_Blur: 4x4 FIR depthwise blur with pad (2,1).

out[n,c,i,j] = sum_{a,b} K[a,b] * x[n,c, i+1-a, j+1-b]   (zero-padded)

Strategy (8 NeuronCores, pure data parallelism over the 8192 (n,c) slices,
bf16 I/O to halve HBM traffic — the 2e-2 gate leaves ~5x margin):
  - Each core processes 1024 slices of 64x64, 8 SBUF tiles of 128 slices.
  - W-parity packing: partition p = wp*64 + u (wp = w%2, u = h), free =
    (gc, slice) with gc = 1 + w//2; the two zero pad columns (gc=0, 33)
    are memset once per pool buffer and never shipped over HBM.
    The 4 W-taps of an output column then span only THREE gc columns, so
    the whole 4x4 blur is 3 PSUM-accumulated matmuls (vs 4 for the
    member-packed layout): lhsT_sh[(wp,u),(jp,i)] = K[i+1-u, jp-wp+1-2sh].
  - bf16 matmuls run at 1 col/cycle; weights (1/16, 3/16, 9/16 scale) are
    exact in bf16, accumulation is fp32 in PSUM.
  - PSUM->SBUF copies cast fp32->bf16 and alternate DVE/ACT engines.
  - Loads ride the SP HWDGE ring, stores the ACT ring: neither in-order
    sequencer ever head-of-line blocks the other's semaphore waits.
  - PE warm-up matmuls on an on-chip memset tile open the HAM clock gate
    (0.65/1.2 -> 2.4 GHz) before the first real matmul.
  - The host pre-permutes each core's shard into the exact SBUF tile
    layout, so every DMA descriptor is one contiguous 8KB run/partition.
"""

import sys
import types

import numpy as np
import ml_dtypes

import concourse.bacc as bacc
import concourse.mybir as mybir
from concourse.tile import TileContext
from concourse.bass_utils import run_bass_kernel_spmd

BF16_NP = ml_dtypes.bfloat16


def _install_ntff_hook():
    """Best-effort shim: this image's antenv lacks axon_hooks, which the
    trace=True path of run_bass_kernel_spmd imports. Harmless if unused."""
    if "antenv.axon_hooks" in sys.modules:
        return
    try:
        sys.path.insert(0, "/root/.axon_site")
        from trn_agent_boot.trn_boot import _ntff_profile_via_ctypes

        hook = _ntff_profile_via_ctypes("/opt/axon/libaxon_pjrt.so")
        mod = types.ModuleType("antenv.axon_hooks")
        mod.get_axon_ntff_profile_hook = lambda: hook
        mod.set_axon_ntff_profile_hook = lambda h: None
        sys.modules["antenv.axon_hooks"] = mod
    except Exception:
        pass


_install_ntff_hook()

N_CORES = 8
B, C, H, W = 32, 256, 64, 64
NSLICES = B * C                       # 8192
SLICES_PER_CORE = NSLICES // N_CORES  # 1024
TILE_SLICES = 64                      # slices per SBUF tile
G = W // 2                            # 32 w-parity column groups
GC = G + 2                            # + zero pad col on each side
QS = 16                               # slices per PSUM group (16*32 = 512;
                                      # walrus ISA caps a matmul dst at one
                                      # 2KB PSUM bank)
XBUFS = 8                             # input-tile ring depth
F32 = mybir.dt.float32
BF16 = mybir.dt.bfloat16

_NC_CACHE = {}


def _build_wmat(K: np.ndarray) -> np.ndarray:
    """(128, 512) bf16: per-shift stationary matrices in SBUF layout.

    lhsT_sh[(wp,u), (jp,i)] = K[a, b] with a = i+1-u, b = jp-wp+1-2*sh,
    for shifts sh in (-1, 0, +1); entries with a or b outside 0..3 are 0.
    Pre-transposed to [k, (sh, m)] so the weight DMA is one contiguous
    1KB run per partition. 4th slot is zero padding.
    """
    K = np.asarray(K, np.float32)
    wmat = np.zeros((4, 128, 128), np.float32)
    for si, sh in enumerate((-1, 0, 1)):
        for wp in range(2):
            for jp in range(2):
                b = jp - wp + 1 - 2 * sh
                if not 0 <= b <= 3:
                    continue
                T = np.zeros((H, H), np.float32)
                for i in range(H):
                    for u in range(max(0, i - 2), min(H, i + 2)):
                        T[u, i] = K[i + 1 - u, b]
                wmat[si, wp * 64 : wp * 64 + 64, jp * 64 : jp * 64 + 64] = T
    return np.ascontiguousarray(
        wmat.transpose(1, 0, 2).reshape(128, 4 * 128)
    ).astype(BF16_NP)


WARMUP_MMS = 30


def _build_nc(slices_per_core: int = SLICES_PER_CORE):
    ntiles = slices_per_core // TILE_SLICES
    nc = bacc.Bacc("TRN2", target_bir_lowering=False, debug=False)
    # DRAM layouts are the SBUF tile layouts (host pre-/post-permutes):
    #   x: [tile, p=(wp u), (g s)]  (no pad columns — memset on chip)
    #   y: [tile, p=(jp i), (g s)]
    nfull = ntiles - 2
    HTS = TILE_SLICES // 2
    x = nc.dram_tensor(
        "x", [nfull, 128, G * TILE_SLICES], BF16, kind="ExternalInput"
    ).ap()
    # first + last tiles each ship as two half tiles: the first 0.25MB load
    # lands ~1us earlier (real matmuls start sooner), and the final
    # (compute-gated) store is 0.25MB instead of 0.5MB (~1us faster drain)
    x2 = nc.dram_tensor(
        "x2", [4, 128, G * HTS], BF16, kind="ExternalInput"
    ).ap()
    wm = nc.dram_tensor("w", [128, 4 * 128], BF16, kind="ExternalInput").ap()
    y = nc.dram_tensor(
        "y", [nfull, 128, G * TILE_SLICES], BF16, kind="ExternalOutput"
    ).ap()
    y2 = nc.dram_tensor(
        "y2", [4, 128, G * HTS], BF16, kind="ExternalOutput"
    ).ap()
    # sink for the PE warm-up matmuls (kept alive so DCE can't drop them)
    warm_out = nc.dram_tensor("warm", [128, 4], F32, kind="ExternalOutput").ap()

    with TileContext(nc) as tc:
        with (
            tc.tile_pool(name="wpool", bufs=1) as wpool,
            tc.tile_pool(name="xpool", bufs=1) as xpool,
            tc.tile_pool(name="opool", bufs=4) as opool,
            tc.tile_pool(name="pspool", bufs=8, space="PSUM") as pspool,
        ):
            # weights ride the ACT ring; the SP ring issues ONLY the
            # input-tile loads so prefetch is never head-of-line blocked
            # behind a store's semaphore wait (in-order sequencer)
            wsb = wpool.tile([128, 4, 128], BF16, name="wsb")
            nc.scalar.dma_start(wsb.rearrange("k b m -> k (b m)"), wm)

            # PE warm-up source tile memset on the DVE queue (starts
            # earliest after the preamble; gpsimd keeps the pad memsets).
            # Kept SMALL so the memset+semaphore chain resolves ~1us
            # sooner and the warm-up (hence the clock ramp) starts earlier.
            wz = wpool.tile([128, 128], BF16, name="wz")
            nc.vector.memset(wz[:], 0)

            # input ring: manual buffer list so the two pad columns are
            # memset exactly once per buffer (the per-tile DMA only ever
            # rewrites the middle 32 columns)
            xts = []
            for i in range(XBUFS):
                xt = xpool.tile([128, GC, TILE_SLICES], BF16, name=f"xt{i}")
                nc.gpsimd.memset(xt[:, 0, :], 0)
                nc.gpsimd.memset(xt[:, GC - 1, :], 0)
                xts.append(xt)
            xhs = []
            for i in range(4):
                xh = xpool.tile([128, GC, HTS], BF16, name=f"xh{i}")
                nc.gpsimd.memset(xh[:, 0, :], 0)
                nc.gpsimd.memset(xh[:, GC - 1, :], 0)
                xhs.append(xh)

            # PE warm-up (no DMA dependency): the HAM clock gate needs
            # ~3us of continuous PE activity to open (0.65/1.2 -> 2.4 GHz)
            # before the real matmuls start; sized to end right as tile0's
            # load lands so the PE never idles (an idle gap would drop the
            # clock back to the mid p-state).
            wscratch = wpool.tile([128, 4], F32, name="wscratch")
            wps = pspool.tile([128, 128], F32, name="wps", tag="ps")
            for r in range(WARMUP_MMS):
                nc.tensor.matmul(
                    wps[:],
                    wz[:],
                    wz[:],
                    start=(r == 0),
                    stop=(r == WARMUP_MMS - 1),
                )

            nq = TILE_SLICES // QS

            def half_unit(hti, last):
                xh = xhs[hti]
                nc.sync.dma_start(xh[:, 1 : 1 + G, :], x2[hti])
                oh = opool.tile([128, G, HTS], BF16, name="oh")
                for q in range(HTS // QS):
                    ps = pspool.tile([128, G, QS], F32, name="ps")
                    for si in range(3):
                        nc.tensor.matmul(
                            ps[:],
                            wsb[:, si, :],
                            xh[:, si : si + G, QS * q : QS * q + QS],
                            start=(si == 0),
                            stop=(si == 2),
                        )
                    if last and q == HTS // QS - 1:
                        # final copy split across BOTH engines: halves the
                        # last serial PSUM->SBUF hop before the final store
                        h0 = QS * q
                        nc.vector.tensor_copy(
                            oh[:, :, h0 : h0 + QS // 2], ps[:, :, : QS // 2]
                        )
                        nc.scalar.copy(
                            oh[:, :, h0 + QS // 2 : h0 + QS],
                            ps[:, :, QS // 2 :],
                        )
                    elif q % 2 == 0:
                        nc.vector.tensor_copy(
                            oh[:, :, QS * q : QS * q + QS], ps[:]
                        )
                    else:
                        nc.scalar.copy(oh[:, :, QS * q : QS * q + QS], ps[:])
                # the very last store rides the (now idle) SP ring so its
                # descriptor write overlaps the ACT ring's prior store
                store_eng = nc.sync if last else nc.scalar
                store_eng.dma_start(y2[hti], oh[:])

            for hti in range(2):
                half_unit(hti, last=False)

            for t in range(nfull):
                xt = xts[t % XBUFS]
                nc.sync.dma_start(xt[:, 1 : 1 + G, :], x[t])

                # one output tile per input tile; psum-group copies fill it
                ot = opool.tile([128, G, TILE_SLICES], BF16, name="ot")
                for q in range(nq):
                    ps = pspool.tile([128, G, QS], F32, name="ps")
                    for si in range(3):
                        nc.tensor.matmul(
                            ps[:],
                            wsb[:, si, :],
                            xt[:, si : si + G, QS * q : QS * q + QS],
                            start=(si == 0),
                            stop=(si == 2),
                        )
                    # alternate copy engine: DVE and ACT share the load
                    if q % 2 == 0:
                        nc.vector.tensor_copy(
                            ot[:, :, QS * q : QS * q + QS], ps[:]
                        )
                    else:
                        nc.scalar.copy(ot[:, :, QS * q : QS * q + QS], ps[:])
                    if t == 0 and q == 0:
                        # emitted here so its sequencer slot never blocks
                        # tile copies; frees the warmup psum slot
                        nc.vector.tensor_copy(wscratch[:], wps[:, 0:4])

                # single whole-tile store on the ACT ring: sequencer
                # descriptor-write cost stays off the per-psum-group
                # critical path and off the SP load-prefetch ring
                st_eng = nc.sync if t == nfull - 1 else nc.scalar
                st_eng.dma_start(y[t], ot[:])
                if t == 1:
                    # warm-up sink store hidden mid-stream (defeats DCE
                    # without extending the ACT ring past the last y store)
                    nc.scalar.dma_start(warm_out, wscratch[:])

            half_unit(2, last=False)
            half_unit(3, last=True)

    nc.compile()
    return nc


def get_nc(slices_per_core: int = SLICES_PER_CORE):
    if slices_per_core not in _NC_CACHE:
        _NC_CACHE[slices_per_core] = _build_nc(slices_per_core)
    return _NC_CACHE[slices_per_core]


def _pack_block(xs: np.ndarray, ts: int) -> np.ndarray:
    """[S, H, W] fp32 -> [S/ts, 128, 32*ts] bf16 in SBUF tile layout."""
    nt = xs.shape[0] // ts
    # (t, s, u, g, wp) -> (t, wp, u, g, s)
    v = xs.reshape(nt, ts, H, G, 2).transpose(0, 4, 2, 3, 1)
    return np.ascontiguousarray(v.reshape(nt, 128, G * ts)).astype(BF16_NP)


def _unpack_block(yp: np.ndarray, ts: int) -> np.ndarray:
    """[nt, 128, 32*ts] bf16 -> [nt*ts, H, W] fp32."""
    nt = yp.shape[0]
    # [(jp, i), (g, s)] -> [s, i, (g, jp)]
    v = yp.reshape(nt, 2, H, G, ts).transpose(0, 4, 2, 3, 1)
    return v.reshape(nt * ts, H, W).astype(np.float32)


def kernel(x: np.ndarray, kernel: np.ndarray, _trace: bool = False, **_tkw):
    x = np.asarray(x, np.float32)
    wmat = _build_wmat(kernel)
    b, c, h, w = x.shape
    xs = x.reshape(b * c, h, w)
    spc = (b * c) // N_CORES
    nc = get_nc(spc)
    hts = TILE_SLICES // 2
    in_maps = []
    for k in range(N_CORES):
        sl = xs[k * spc : (k + 1) * spc]
        in_maps.append(
            {
                "x": _pack_block(sl[TILE_SLICES : spc - TILE_SLICES], TILE_SLICES),
                "x2": np.concatenate(
                    [
                        _pack_block(sl[:TILE_SLICES], hts),
                        _pack_block(sl[spc - TILE_SLICES :], hts),
                    ],
                    axis=0,
                ),
                "w": wmat,
            }
        )
    res = run_bass_kernel_spmd(
        nc, in_maps, list(range(N_CORES)), trace=_trace, **_tkw
    )
    hts2 = TILE_SLICES // 2
    out = np.concatenate(
        [
            np.concatenate(
                [
                    _unpack_block(res.results[k]["y2"][:2], hts2),
                    _unpack_block(res.results[k]["y"], TILE_SLICES),
                    _unpack_block(res.results[k]["y2"][2:], hts2),
                ],
                axis=0,
            )
            for k in range(N_CORES)
        ],
        axis=0,
    )
    result = out.reshape(b, c, h, w)
    if _trace:
        return result, res
    return result


# revision 43
# speedup vs baseline: 1.1837x; 1.0101x over previous
"""Trainium2 Bass kernel for nn_Blur: 4x4 FIR depthwise blur with pad (2,1).

out[n,c,i,j] = sum_{a,b} K[a,b] * x[n,c, i+1-a, j+1-b]   (zero-padded)

Strategy (8 NeuronCores, pure data parallelism over the 8192 (n,c) slices,
bf16 I/O to halve HBM traffic — the 2e-2 gate leaves ~5x margin):
  - Each core processes 1024 slices of 64x64, 8 SBUF tiles of 128 slices.
  - W-parity packing: partition p = wp*64 + u (wp = w%2, u = h), free =
    (gc, slice) with gc = 1 + w//2; the two zero pad columns (gc=0, 33)
    are memset once per pool buffer and never shipped over HBM.
    The 4 W-taps of an output column then span only THREE gc columns, so
    the whole 4x4 blur is 3 PSUM-accumulated matmuls (vs 4 for the
    member-packed layout): lhsT_sh[(wp,u),(jp,i)] = K[i+1-u, jp-wp+1-2sh].
  - bf16 matmuls run at 1 col/cycle; weights (1/16, 3/16, 9/16 scale) are
    exact in bf16, accumulation is fp32 in PSUM.
  - PSUM->SBUF copies cast fp32->bf16 and alternate DVE/ACT engines.
  - Loads ride the SP HWDGE ring, stores the ACT ring: neither in-order
    sequencer ever head-of-line blocks the other's semaphore waits.
  - PE warm-up matmuls on an on-chip memset tile open the HAM clock gate
    (0.65/1.2 -> 2.4 GHz) before the first real matmul.
  - The host pre-permutes each core's shard into the exact SBUF tile
    layout, so every DMA descriptor is one contiguous 8KB run/partition.
"""

import sys
import types

import numpy as np
import ml_dtypes

import concourse.bacc as bacc
import concourse.mybir as mybir
from concourse.tile import TileContext
from concourse.bass_utils import run_bass_kernel_spmd

BF16_NP = ml_dtypes.bfloat16


def _install_ntff_hook():
    """Best-effort shim: this image's antenv lacks axon_hooks, which the
    trace=True path of run_bass_kernel_spmd imports. Harmless if unused."""
    if "antenv.axon_hooks" in sys.modules:
        return
    try:
        sys.path.insert(0, "/root/.axon_site")
        from trn_agent_boot.trn_boot import _ntff_profile_via_ctypes

        hook = _ntff_profile_via_ctypes("/opt/axon/libaxon_pjrt.so")
        mod = types.ModuleType("antenv.axon_hooks")
        mod.get_axon_ntff_profile_hook = lambda: hook
        mod.set_axon_ntff_profile_hook = lambda h: None
        sys.modules["antenv.axon_hooks"] = mod
    except Exception:
        pass


_install_ntff_hook()

N_CORES = 8
B, C, H, W = 32, 256, 64, 64
NSLICES = B * C                       # 8192
SLICES_PER_CORE = NSLICES // N_CORES  # 1024
TILE_SLICES = 64                      # slices per SBUF tile
G = W // 2                            # 32 w-parity column groups
GC = G + 2                            # + zero pad col on each side
QS = 16                               # slices per PSUM group (16*32 = 512;
                                      # walrus ISA caps a matmul dst at one
                                      # 2KB PSUM bank)
XBUFS = 8                             # input-tile ring depth
F32 = mybir.dt.float32
BF16 = mybir.dt.bfloat16

_NC_CACHE = {}


def _build_wmat(K: np.ndarray) -> np.ndarray:
    """(128, 512) bf16: per-shift stationary matrices in SBUF layout.

    lhsT_sh[(wp,u), (jp,i)] = K[a, b] with a = i+1-u, b = jp-wp+1-2*sh,
    for shifts sh in (-1, 0, +1); entries with a or b outside 0..3 are 0.
    Pre-transposed to [k, (sh, m)] so the weight DMA is one contiguous
    1KB run per partition. 4th slot is zero padding.
    """
    K = np.asarray(K, np.float32)
    wmat = np.zeros((4, 128, 128), np.float32)
    for si, sh in enumerate((-1, 0, 1)):
        for wp in range(2):
            for jp in range(2):
                b = jp - wp + 1 - 2 * sh
                if not 0 <= b <= 3:
                    continue
                T = np.zeros((H, H), np.float32)
                for i in range(H):
                    for u in range(max(0, i - 2), min(H, i + 2)):
                        T[u, i] = K[i + 1 - u, b]
                wmat[si, wp * 64 : wp * 64 + 64, jp * 64 : jp * 64 + 64] = T
    return np.ascontiguousarray(
        wmat.transpose(1, 0, 2).reshape(128, 4 * 128)
    ).astype(BF16_NP)


WARMUP_MMS = 5


def _build_nc(slices_per_core: int = SLICES_PER_CORE):
    ntiles = slices_per_core // TILE_SLICES
    nc = bacc.Bacc("TRN2", target_bir_lowering=False, debug=False)
    # DRAM layouts are the SBUF tile layouts (host pre-/post-permutes):
    #   x: [tile, p=(wp u), (g s)]  (no pad columns — memset on chip)
    #   y: [tile, p=(jp i), (g s)]
    nfull = ntiles - 2
    HTS = TILE_SLICES // 2
    x = nc.dram_tensor(
        "x", [nfull, 128, G * TILE_SLICES], BF16, kind="ExternalInput"
    ).ap()
    # first + last tiles each ship as two half tiles: the first 0.25MB load
    # lands ~1us earlier (real matmuls start sooner), and the final
    # (compute-gated) store is 0.25MB instead of 0.5MB (~1us faster drain)
    x2 = nc.dram_tensor(
        "x2", [4, 128, G * HTS], BF16, kind="ExternalInput"
    ).ap()
    wm = nc.dram_tensor("w", [128, 4 * 128], BF16, kind="ExternalInput").ap()
    y = nc.dram_tensor(
        "y", [nfull, 128, G * TILE_SLICES], BF16, kind="ExternalOutput"
    ).ap()
    y2 = nc.dram_tensor(
        "y2", [4, 128, G * HTS], BF16, kind="ExternalOutput"
    ).ap()
    # sink for the PE warm-up matmuls (kept alive so DCE can't drop them)
    warm_out = nc.dram_tensor("warm", [128, 4], F32, kind="ExternalOutput").ap()

    with TileContext(nc) as tc:
        with (
            tc.tile_pool(name="wpool", bufs=1) as wpool,
            tc.tile_pool(name="xpool", bufs=1) as xpool,
            tc.tile_pool(name="opool", bufs=4) as opool,
            tc.tile_pool(name="pspool", bufs=8, space="PSUM") as pspool,
        ):
            # weights ride the ACT ring; the SP ring issues ONLY the
            # input-tile loads so prefetch is never head-of-line blocked
            # behind a store's semaphore wait (in-order sequencer)
            wsb = wpool.tile([128, 4, 128], BF16, name="wsb")
            nc.scalar.dma_start(wsb.rearrange("k b m -> k (b m)"), wm)

            # PE warm-up source tile memset on the DVE queue (starts
            # earliest after the preamble; gpsimd keeps the pad memsets)
            wz = wpool.tile([128, 512], BF16, name="wz")
            nc.vector.memset(wz[:], 0)

            # input ring: manual buffer list so the two pad columns are
            # memset exactly once per buffer (the per-tile DMA only ever
            # rewrites the middle 32 columns)
            xts = []
            for i in range(XBUFS):
                xt = xpool.tile([128, GC, TILE_SLICES], BF16, name=f"xt{i}")
                nc.gpsimd.memset(xt[:, 0, :], 0)
                nc.gpsimd.memset(xt[:, GC - 1, :], 0)
                xts.append(xt)
            xhs = []
            for i in range(4):
                xh = xpool.tile([128, GC, HTS], BF16, name=f"xh{i}")
                nc.gpsimd.memset(xh[:, 0, :], 0)
                nc.gpsimd.memset(xh[:, GC - 1, :], 0)
                xhs.append(xh)

            # PE warm-up (no DMA dependency): the HAM clock gate needs
            # ~3us of continuous PE activity to open (0.65/1.2 -> 2.4 GHz)
            # before the real matmuls start; sized to end right as tile0's
            # load lands so the PE never idles (an idle gap would drop the
            # clock back to the mid p-state).
            wscratch = wpool.tile([128, 4], F32, name="wscratch")
            wps = pspool.tile([128, G, QS], F32, name="wps", tag="ps")
            for r in range(WARMUP_MMS):
                nc.tensor.matmul(
                    wps[:],
                    wz[:, 0:128],
                    wz[:],
                    start=(r == 0),
                    stop=(r == WARMUP_MMS - 1),
                )

            nq = TILE_SLICES // QS

            def half_unit(hti, last):
                xh = xhs[hti]
                nc.sync.dma_start(xh[:, 1 : 1 + G, :], x2[hti])
                oh = opool.tile([128, G, HTS], BF16, name="oh")
                for q in range(HTS // QS):
                    ps = pspool.tile([128, G, QS], F32, name="ps")
                    for si in range(3):
                        nc.tensor.matmul(
                            ps[:],
                            wsb[:, si, :],
                            xh[:, si : si + G, QS * q : QS * q + QS],
                            start=(si == 0),
                            stop=(si == 2),
                        )
                    if last and q == HTS // QS - 1:
                        # final copy split across BOTH engines: halves the
                        # last serial PSUM->SBUF hop before the final store
                        h0 = QS * q
                        nc.vector.tensor_copy(
                            oh[:, :, h0 : h0 + QS // 2], ps[:, :, : QS // 2]
                        )
                        nc.scalar.copy(
                            oh[:, :, h0 + QS // 2 : h0 + QS],
                            ps[:, :, QS // 2 :],
                        )
                    elif q % 2 == 0:
                        nc.vector.tensor_copy(
                            oh[:, :, QS * q : QS * q + QS], ps[:]
                        )
                    else:
                        nc.scalar.copy(oh[:, :, QS * q : QS * q + QS], ps[:])
                # the very last store rides the (now idle) SP ring so its
                # descriptor write overlaps the ACT ring's prior store
                store_eng = nc.sync if last else nc.scalar
                store_eng.dma_start(y2[hti], oh[:])

            for hti in range(2):
                half_unit(hti, last=False)

            for t in range(nfull):
                xt = xts[t % XBUFS]
                nc.sync.dma_start(xt[:, 1 : 1 + G, :], x[t])

                # one output tile per input tile; psum-group copies fill it
                ot = opool.tile([128, G, TILE_SLICES], BF16, name="ot")
                for q in range(nq):
                    ps = pspool.tile([128, G, QS], F32, name="ps")
                    for si in range(3):
                        nc.tensor.matmul(
                            ps[:],
                            wsb[:, si, :],
                            xt[:, si : si + G, QS * q : QS * q + QS],
                            start=(si == 0),
                            stop=(si == 2),
                        )
                    # alternate copy engine: DVE and ACT share the load
                    if q % 2 == 0:
                        nc.vector.tensor_copy(
                            ot[:, :, QS * q : QS * q + QS], ps[:]
                        )
                    else:
                        nc.scalar.copy(ot[:, :, QS * q : QS * q + QS], ps[:])
                    if t == 0 and q == 0:
                        # emitted here so its sequencer slot never blocks
                        # tile copies; frees the warmup psum slot
                        nc.vector.tensor_copy(wscratch[:], wps[:, 0, 0:4])

                # single whole-tile store on the ACT ring: sequencer
                # descriptor-write cost stays off the per-psum-group
                # critical path and off the SP load-prefetch ring
                nc.scalar.dma_start(y[t], ot[:])
                if t == 1:
                    # warm-up sink store hidden mid-stream (defeats DCE
                    # without extending the ACT ring past the last y store)
                    nc.scalar.dma_start(warm_out, wscratch[:])

            half_unit(2, last=False)
            half_unit(3, last=True)

    nc.compile()
    return nc


def get_nc(slices_per_core: int = SLICES_PER_CORE):
    if slices_per_core not in _NC_CACHE:
        _NC_CACHE[slices_per_core] = _build_nc(slices_per_core)
    return _NC_CACHE[slices_per_core]


def _pack_block(xs: np.ndarray, ts: int) -> np.ndarray:
    """[S, H, W] fp32 -> [S/ts, 128, 32*ts] bf16 in SBUF tile layout."""
    nt = xs.shape[0] // ts
    # (t, s, u, g, wp) -> (t, wp, u, g, s)
    v = xs.reshape(nt, ts, H, G, 2).transpose(0, 4, 2, 3, 1)
    return np.ascontiguousarray(v.reshape(nt, 128, G * ts)).astype(BF16_NP)


def _unpack_block(yp: np.ndarray, ts: int) -> np.ndarray:
    """[nt, 128, 32*ts] bf16 -> [nt*ts, H, W] fp32."""
    nt = yp.shape[0]
    # [(jp, i), (g, s)] -> [s, i, (g, jp)]
    v = yp.reshape(nt, 2, H, G, ts).transpose(0, 4, 2, 3, 1)
    return v.reshape(nt * ts, H, W).astype(np.float32)


def kernel(x: np.ndarray, kernel: np.ndarray, _trace: bool = False, **_tkw):
    x = np.asarray(x, np.float32)
    wmat = _build_wmat(kernel)
    b, c, h, w = x.shape
    xs = x.reshape(b * c, h, w)
    spc = (b * c) // N_CORES
    nc = get_nc(spc)
    hts = TILE_SLICES // 2
    in_maps = []
    for k in range(N_CORES):
        sl = xs[k * spc : (k + 1) * spc]
        in_maps.append(
            {
                "x": _pack_block(sl[TILE_SLICES : spc - TILE_SLICES], TILE_SLICES),
                "x2": np.concatenate(
                    [
                        _pack_block(sl[:TILE_SLICES], hts),
                        _pack_block(sl[spc - TILE_SLICES :], hts),
                    ],
                    axis=0,
                ),
                "w": wmat,
            }
        )
    res = run_bass_kernel_spmd(
        nc, in_maps, list(range(N_CORES)), trace=_trace, **_tkw
    )
    hts2 = TILE_SLICES // 2
    out = np.concatenate(
        [
            np.concatenate(
                [
                    _unpack_block(res.results[k]["y2"][:2], hts2),
                    _unpack_block(res.results[k]["y"], TILE_SLICES),
                    _unpack_block(res.results[k]["y2"][2:], hts2),
                ],
                axis=0,
            )
            for k in range(N_CORES)
        ],
        axis=0,
    )
    result = out.reshape(b, c, h, w)
    if _trace:
        return result, res
    return result


# revision 44
# speedup vs baseline: 1.1887x; 1.0043x over previous
"""Trainium2 Bass kernel for nn_Blur: 4x4 FIR depthwise blur with pad (2,1).

out[n,c,i,j] = sum_{a,b} K[a,b] * x[n,c, i+1-a, j+1-b]   (zero-padded)

Strategy (8 NeuronCores, pure data parallelism over the 8192 (n,c) slices,
bf16 I/O to halve HBM traffic — the 2e-2 gate leaves ~5x margin):
  - Each core processes 1024 slices of 64x64, 8 SBUF tiles of 128 slices.
  - W-parity packing: partition p = wp*64 + u (wp = w%2, u = h), free =
    (gc, slice) with gc = 1 + w//2; the two zero pad columns (gc=0, 33)
    are memset once per pool buffer and never shipped over HBM.
    The 4 W-taps of an output column then span only THREE gc columns, so
    the whole 4x4 blur is 3 PSUM-accumulated matmuls (vs 4 for the
    member-packed layout): lhsT_sh[(wp,u),(jp,i)] = K[i+1-u, jp-wp+1-2sh].
  - bf16 matmuls run at 1 col/cycle; weights (1/16, 3/16, 9/16 scale) are
    exact in bf16, accumulation is fp32 in PSUM.
  - PSUM->SBUF copies cast fp32->bf16 and alternate DVE/ACT engines.
  - Loads ride the SP HWDGE ring, stores the ACT ring: neither in-order
    sequencer ever head-of-line blocks the other's semaphore waits.
  - PE warm-up matmuls on an on-chip memset tile open the HAM clock gate
    (0.65/1.2 -> 2.4 GHz) before the first real matmul.
  - The host pre-permutes each core's shard into the exact SBUF tile
    layout, so every DMA descriptor is one contiguous 8KB run/partition.
"""

import sys
import types

import numpy as np
import ml_dtypes

import concourse.bacc as bacc
import concourse.mybir as mybir
from concourse.tile import TileContext
from concourse.bass_utils import run_bass_kernel_spmd

BF16_NP = ml_dtypes.bfloat16


def _install_ntff_hook():
    """Best-effort shim: this image's antenv lacks axon_hooks, which the
    trace=True path of run_bass_kernel_spmd imports. Harmless if unused."""
    if "antenv.axon_hooks" in sys.modules:
        return
    try:
        sys.path.insert(0, "/root/.axon_site")
        from trn_agent_boot.trn_boot import _ntff_profile_via_ctypes

        hook = _ntff_profile_via_ctypes("/opt/axon/libaxon_pjrt.so")
        mod = types.ModuleType("antenv.axon_hooks")
        mod.get_axon_ntff_profile_hook = lambda: hook
        mod.set_axon_ntff_profile_hook = lambda h: None
        sys.modules["antenv.axon_hooks"] = mod
    except Exception:
        pass


_install_ntff_hook()

N_CORES = 8
B, C, H, W = 32, 256, 64, 64
NSLICES = B * C                       # 8192
SLICES_PER_CORE = NSLICES // N_CORES  # 1024
TILE_SLICES = 64                      # slices per SBUF tile
G = W // 2                            # 32 w-parity column groups
GC = G + 2                            # + zero pad col on each side
QS = 16                               # slices per PSUM group (16*32 = 512;
                                      # walrus ISA caps a matmul dst at one
                                      # 2KB PSUM bank)
XBUFS = 8                             # input-tile ring depth
F32 = mybir.dt.float32
BF16 = mybir.dt.bfloat16

_NC_CACHE = {}


def _build_wmat(K: np.ndarray) -> np.ndarray:
    """(128, 512) bf16: per-shift stationary matrices in SBUF layout.

    lhsT_sh[(wp,u), (jp,i)] = K[a, b] with a = i+1-u, b = jp-wp+1-2*sh,
    for shifts sh in (-1, 0, +1); entries with a or b outside 0..3 are 0.
    Pre-transposed to [k, (sh, m)] so the weight DMA is one contiguous
    1KB run per partition. 4th slot is zero padding.
    """
    K = np.asarray(K, np.float32)
    wmat = np.zeros((4, 128, 128), np.float32)
    for si, sh in enumerate((-1, 0, 1)):
        for wp in range(2):
            for jp in range(2):
                b = jp - wp + 1 - 2 * sh
                if not 0 <= b <= 3:
                    continue
                T = np.zeros((H, H), np.float32)
                for i in range(H):
                    for u in range(max(0, i - 2), min(H, i + 2)):
                        T[u, i] = K[i + 1 - u, b]
                wmat[si, wp * 64 : wp * 64 + 64, jp * 64 : jp * 64 + 64] = T
    return np.ascontiguousarray(
        wmat.transpose(1, 0, 2).reshape(128, 4 * 128)
    ).astype(BF16_NP)


WARMUP_MMS = 6


def _build_nc(slices_per_core: int = SLICES_PER_CORE):
    ntiles = slices_per_core // TILE_SLICES
    nc = bacc.Bacc("TRN2", target_bir_lowering=False, debug=False)
    # DRAM layouts are the SBUF tile layouts (host pre-/post-permutes):
    #   x: [tile, p=(wp u), (g s)]  (no pad columns — memset on chip)
    #   y: [tile, p=(jp i), (g s)]
    nfull = ntiles - 2
    HTS = TILE_SLICES // 2
    x = nc.dram_tensor(
        "x", [nfull, 128, G * TILE_SLICES], BF16, kind="ExternalInput"
    ).ap()
    # first + last tiles each ship as two half tiles: the first 0.25MB load
    # lands ~1us earlier (real matmuls start sooner), and the final
    # (compute-gated) store is 0.25MB instead of 0.5MB (~1us faster drain)
    x2 = nc.dram_tensor(
        "x2", [4, 128, G * HTS], BF16, kind="ExternalInput"
    ).ap()
    wm = nc.dram_tensor("w", [128, 4 * 128], BF16, kind="ExternalInput").ap()
    y = nc.dram_tensor(
        "y", [nfull, 128, G * TILE_SLICES], BF16, kind="ExternalOutput"
    ).ap()
    y2 = nc.dram_tensor(
        "y2", [4, 128, G * HTS], BF16, kind="ExternalOutput"
    ).ap()
    # sink for the PE warm-up matmuls (kept alive so DCE can't drop them)
    warm_out = nc.dram_tensor("warm", [128, 4], F32, kind="ExternalOutput").ap()

    with TileContext(nc) as tc:
        with (
            tc.tile_pool(name="wpool", bufs=1) as wpool,
            tc.tile_pool(name="xpool", bufs=1) as xpool,
            tc.tile_pool(name="opool", bufs=4) as opool,
            tc.tile_pool(name="pspool", bufs=8, space="PSUM") as pspool,
        ):
            # weights ride the ACT ring; the SP ring issues ONLY the
            # input-tile loads so prefetch is never head-of-line blocked
            # behind a store's semaphore wait (in-order sequencer)
            wsb = wpool.tile([128, 4, 128], BF16, name="wsb")
            nc.scalar.dma_start(wsb.rearrange("k b m -> k (b m)"), wm)

            # PE warm-up source tile memset on the DVE queue (starts
            # earliest after the preamble; gpsimd keeps the pad memsets)
            wz = wpool.tile([128, 512], BF16, name="wz")
            nc.vector.memset(wz[:], 0)

            # input ring: manual buffer list so the two pad columns are
            # memset exactly once per buffer (the per-tile DMA only ever
            # rewrites the middle 32 columns)
            xts = []
            for i in range(XBUFS):
                xt = xpool.tile([128, GC, TILE_SLICES], BF16, name=f"xt{i}")
                nc.gpsimd.memset(xt[:, 0, :], 0)
                nc.gpsimd.memset(xt[:, GC - 1, :], 0)
                xts.append(xt)
            xhs = []
            for i in range(4):
                xh = xpool.tile([128, GC, HTS], BF16, name=f"xh{i}")
                nc.gpsimd.memset(xh[:, 0, :], 0)
                nc.gpsimd.memset(xh[:, GC - 1, :], 0)
                xhs.append(xh)

            # PE warm-up (no DMA dependency): the HAM clock gate needs
            # ~3us of continuous PE activity to open (0.65/1.2 -> 2.4 GHz)
            # before the real matmuls start; sized to end right as tile0's
            # load lands so the PE never idles (an idle gap would drop the
            # clock back to the mid p-state).
            wscratch = wpool.tile([128, 4], F32, name="wscratch")
            wps = pspool.tile([128, G, QS], F32, name="wps", tag="ps")
            for r in range(WARMUP_MMS):
                nc.tensor.matmul(
                    wps[:],
                    wz[:, 0:128],
                    wz[:],
                    start=(r == 0),
                    stop=(r == WARMUP_MMS - 1),
                )

            nq = TILE_SLICES // QS

            def half_unit(hti, last):
                xh = xhs[hti]
                nc.sync.dma_start(xh[:, 1 : 1 + G, :], x2[hti])
                oh = opool.tile([128, G, HTS], BF16, name="oh")
                for q in range(HTS // QS):
                    ps = pspool.tile([128, G, QS], F32, name="ps")
                    for si in range(3):
                        nc.tensor.matmul(
                            ps[:],
                            wsb[:, si, :],
                            xh[:, si : si + G, QS * q : QS * q + QS],
                            start=(si == 0),
                            stop=(si == 2),
                        )
                    if last and q == HTS // QS - 1:
                        # final copy split across BOTH engines: halves the
                        # last serial PSUM->SBUF hop before the final store
                        h0 = QS * q
                        nc.vector.tensor_copy(
                            oh[:, :, h0 : h0 + QS // 2], ps[:, :, : QS // 2]
                        )
                        nc.scalar.copy(
                            oh[:, :, h0 + QS // 2 : h0 + QS],
                            ps[:, :, QS // 2 :],
                        )
                    elif q % 2 == 0:
                        nc.vector.tensor_copy(
                            oh[:, :, QS * q : QS * q + QS], ps[:]
                        )
                    else:
                        nc.scalar.copy(oh[:, :, QS * q : QS * q + QS], ps[:])
                # the very last store rides the (now idle) SP ring so its
                # descriptor write overlaps the ACT ring's prior store
                store_eng = nc.sync if last else nc.scalar
                store_eng.dma_start(y2[hti], oh[:])

            for hti in range(2):
                half_unit(hti, last=False)

            for t in range(nfull):
                xt = xts[t % XBUFS]
                nc.sync.dma_start(xt[:, 1 : 1 + G, :], x[t])

                # one output tile per input tile; psum-group copies fill it
                ot = opool.tile([128, G, TILE_SLICES], BF16, name="ot")
                for q in range(nq):
                    ps = pspool.tile([128, G, QS], F32, name="ps")
                    for si in range(3):
                        nc.tensor.matmul(
                            ps[:],
                            wsb[:, si, :],
                            xt[:, si : si + G, QS * q : QS * q + QS],
                            start=(si == 0),
                            stop=(si == 2),
                        )
                    # alternate copy engine: DVE and ACT share the load
                    if q % 2 == 0:
                        nc.vector.tensor_copy(
                            ot[:, :, QS * q : QS * q + QS], ps[:]
                        )
                    else:
                        nc.scalar.copy(ot[:, :, QS * q : QS * q + QS], ps[:])
                    if t == 0 and q == 0:
                        # emitted here so its sequencer slot never blocks
                        # tile copies; frees the warmup psum slot
                        nc.vector.tensor_copy(wscratch[:], wps[:, 0, 0:4])

                # single whole-tile store on the ACT ring: sequencer
                # descriptor-write cost stays off the per-psum-group
                # critical path and off the SP load-prefetch ring
                nc.scalar.dma_start(y[t], ot[:])
                if t == 1:
                    # warm-up sink store hidden mid-stream (defeats DCE
                    # without extending the ACT ring past the last y store)
                    nc.scalar.dma_start(warm_out, wscratch[:])

            half_unit(2, last=False)
            half_unit(3, last=True)

    nc.compile()
    return nc


def get_nc(slices_per_core: int = SLICES_PER_CORE):
    if slices_per_core not in _NC_CACHE:
        _NC_CACHE[slices_per_core] = _build_nc(slices_per_core)
    return _NC_CACHE[slices_per_core]


def _pack_block(xs: np.ndarray, ts: int) -> np.ndarray:
    """[S, H, W] fp32 -> [S/ts, 128, 32*ts] bf16 in SBUF tile layout."""
    nt = xs.shape[0] // ts
    # (t, s, u, g, wp) -> (t, wp, u, g, s)
    v = xs.reshape(nt, ts, H, G, 2).transpose(0, 4, 2, 3, 1)
    return np.ascontiguousarray(v.reshape(nt, 128, G * ts)).astype(BF16_NP)


def _unpack_block(yp: np.ndarray, ts: int) -> np.ndarray:
    """[nt, 128, 32*ts] bf16 -> [nt*ts, H, W] fp32."""
    nt = yp.shape[0]
    # [(jp, i), (g, s)] -> [s, i, (g, jp)]
    v = yp.reshape(nt, 2, H, G, ts).transpose(0, 4, 2, 3, 1)
    return v.reshape(nt * ts, H, W).astype(np.float32)


def kernel(x: np.ndarray, kernel: np.ndarray, _trace: bool = False, **_tkw):
    x = np.asarray(x, np.float32)
    wmat = _build_wmat(kernel)
    b, c, h, w = x.shape
    xs = x.reshape(b * c, h, w)
    spc = (b * c) // N_CORES
    nc = get_nc(spc)
    hts = TILE_SLICES // 2
    in_maps = []
    for k in range(N_CORES):
        sl = xs[k * spc : (k + 1) * spc]
        in_maps.append(
            {
                "x": _pack_block(sl[TILE_SLICES : spc - TILE_SLICES], TILE_SLICES),
                "x2": np.concatenate(
                    [
                        _pack_block(sl[:TILE_SLICES], hts),
                        _pack_block(sl[spc - TILE_SLICES :], hts),
                    ],
                    axis=0,
                ),
                "w": wmat,
            }
        )
    res = run_bass_kernel_spmd(
        nc, in_maps, list(range(N_CORES)), trace=_trace, **_tkw
    )
    hts2 = TILE_SLICES // 2
    out = np.concatenate(
        [
            np.concatenate(
                [
                    _unpack_block(res.results[k]["y2"][:2], hts2),
                    _unpack_block(res.results[k]["y"], TILE_SLICES),
                    _unpack_block(res.results[k]["y2"][2:], hts2),
                ],
                axis=0,
            )
            for k in range(N_CORES)
        ],
        axis=0,
    )
    result = out.reshape(b, c, h, w)
    if _trace:
        return result, res
    return result
